# revision 12
# baseline (speedup 1.0000x reference)
"""MultiHeadAttention Trainium2 kernel (8 NeuronCores).

Problem: B=2, S=2048, E=1024, H=16, HD=64.
  qg = q @ Wq + bq ; qh[h] = qg @ Whq[h] + bhq[h]   (same for k, v)
  scores = qh @ kh^T / sqrt(HD), masked (-inf where mask), softmax
  out = concat_h(softmax @ vh) @ Wo + bo

Sharding: core c = 4*b + g handles batch b, heads 4g..4g+3 (data parallel on
B, tensor parallel on H). The global+per-head projections are folded on the
host into per-head fused weights Fq[h] = Wq @ Whq[h] (etc.), so each core
runs one [E, HD] projection per head. The output projection is row-sharded:
each core computes Wo[256g:256g+256]^T @ x^T and the host sums partials.

Data path is bf16 (matches the PE's full-rate mode + FWL weight loads +
DVE 2x mode); all matmul accumulation is fp32 in PSUM, and the softmax
normalization (reciprocal of denominators) runs in f32r.

On-chip layout (per core):
  qT/kT/vT [E=1024, S=2048] bf16  (host pre-transposed)
  qhT/khT: two [128, 2048] tiles, heads (2p, 2p+1) stacked on partition
    halves -> QK matmuls for adjacent heads land on disjoint PE row groups.
  vh: 16 tiles [128 (k-chunk), 4*65]; per head 64 value cols + a fused
    ones column (projection with zero weights, bias 1) so the AV matmul
    also produces softmax denominators.
  scores kept transposed [k, q]: softmax reduction becomes a matmul
    contraction; the probability tile is directly the AV moving operand.
  Per (q-chunk, k-chunk): one [128, 1024] PSUM scores tile per head-pair,
  one exp over the pair, per-head mask multiply (keep-mask, post-exp),
  AV accumulate into per-head [65, 512] PSUM banks.
"""
import ml_dtypes
import numpy as np
from contextlib import ExitStack

import concourse.bass as bass
import concourse.mybir as mybir
import concourse.tile as tile
from concourse import bacc

dt = mybir.dt
AF = mybir.ActivationFunctionType
OP = mybir.AluOpType

B, S, E, H = 2, 2048, 1024, 16
HD = E // H          # 64
HPC = H // 4         # heads per core = 4
N_CORES = 8
ECH = E // 128       # 8 e-chunks
NQ = S // 512        # 4 q chunks
NK = S // 128        # 16 k chunks

_prog_cache = {}


def build_program():
    if "nc" in _prog_cache:
        return _prog_cache["nc"]
    nc = bacc.Bacc("TRN2", target_bir_lowering=False, debug=False,
                   num_devices=N_CORES)

    bf = dt.bfloat16
    qT = nc.dram_tensor("qT", [E, S], bf, kind="ExternalInput").ap()
    kT = nc.dram_tensor("kT", [E, S], bf, kind="ExternalInput").ap()
    vT = nc.dram_tensor("vT", [E, S], bf, kind="ExternalInput").ap()
    maskT = nc.dram_tensor("maskT", [S, S], bf, kind="ExternalInput").ap()
    Fq = nc.dram_tensor("Fq", [E, 256], bf, kind="ExternalInput").ap()
    Fk = nc.dram_tensor("Fk", [E, 256], bf, kind="ExternalInput").ap()
    Fv = nc.dram_tensor("Fv", [E, 260], bf, kind="ExternalInput").ap()
    bfq = nc.dram_tensor("bfq", [128, 2], dt.float32, kind="ExternalInput").ap()
    bfk = nc.dram_tensor("bfk", [128, 2], dt.float32, kind="ExternalInput").ap()
    bfv = nc.dram_tensor("bfv", [1, 260], bf, kind="ExternalInput").ap()
    Wo = nc.dram_tensor("Wo", [256, 1024], bf, kind="ExternalInput").ap()
    onesb = nc.dram_tensor("onesb", [1, 128], bf, kind="ExternalInput").ap()
    # head-selector for denominator broadcast: row h ones in cols 64h..64h+63
    sel = nc.dram_tensor("sel", [128, 256], dt.float32r, kind="ExternalInput").ap()
    out_pT = nc.dram_tensor("out_pT", [E, S], dt.float32, kind="ExternalOutput").ap()
    warm_out = nc.dram_tensor("warm_out", [128, 512], dt.float32, kind="ExternalOutput").ap()

    with tile.TileContext(nc) as tc:
        with ExitStack() as ctx:
            wc = ctx.enter_context(tc.tile_pool(name="wc", bufs=1))
            xin = ctx.enter_context(tc.tile_pool(name="xin", bufs=12))
            qk = ctx.enter_context(tc.tile_pool(name="qk", bufs=1))
            vhp = ctx.enter_context(tc.tile_pool(name="vhp", bufs=1))
            xTp = ctx.enter_context(tc.tile_pool(name="xTp", bufs=1))
            maskp = ctx.enter_context(tc.tile_pool(name="maskp", bufs=6))
            escp = ctx.enter_context(tc.tile_pool(name="escp", bufs=8))
            avnp = ctx.enter_context(tc.tile_pool(name="avnp", bufs=2))
            oev = ctx.enter_context(tc.tile_pool(name="oev", bufs=2))

            # ---- constants ----
            Fq_sb = [wc.tile([128, 256], bf, tag=f"Fq{e}", name=f"Fq{e}") for e in range(ECH)]
            Fk_sb = [wc.tile([128, 256], bf, tag=f"Fk{e}", name=f"Fk{e}") for e in range(ECH)]
            Fv_sb = [wc.tile([128, 260], bf, tag=f"Fv{e}", name=f"Fv{e}") for e in range(ECH)]
            for e in range(ECH):
                nc.sync.dma_start(Fq_sb[e][:], Fq[bass.ts(e, 128), :])
                nc.sync.dma_start(Fk_sb[e][:], Fk[bass.ts(e, 128), :])
                nc.sync.dma_start(Fv_sb[e][:], Fv[bass.ts(e, 128), :])
            Wo_sb = [wc.tile([128, 1024], bf, tag=f"Wo{c}", name=f"Wo{c}") for c in range(2)]
            for c in range(2):
                nc.sync.dma_start(Wo_sb[c][:], Wo[bass.ts(c, 128), :])
            bfq_sb = wc.tile([128, 2], dt.float32, tag="bfq")
            bfk_sb = wc.tile([128, 2], dt.float32, tag="bfk")
            bfv_sb = wc.tile([1, 260], bf, tag="bfv")
            onesb_sb = wc.tile([1, 128], bf, tag="onesb")
            sel_sb = wc.tile([128, 256], dt.float32r, tag="sel")
            nc.sync.dma_start(bfq_sb[:], bfq)
            nc.sync.dma_start(bfk_sb[:], bfk)
            nc.sync.dma_start(bfv_sb[:], bfv)
            nc.sync.dma_start(onesb_sb[:], onesb)
            nc.sync.dma_start(sel_sb[:], sel)

            qhT = [qk.tile([128, S], bf, tag=f"qhT{p}", name=f"qhT{p}") for p in range(2)]
            khT = [qk.tile([128, S], bf, tag=f"khT{p}", name=f"khT{p}") for p in range(2)]
            vh_sb = [vhp.tile([128, 4 * 65], bf, tag=f"vh{sc}", name=f"vh{sc}") for sc in range(NK)]
            xT_sb = [xTp.tile([128, S], bf, tag=f"xT{c}", name=f"xT{c}") for c in range(2)]

            # ---- phase 0: PE warm-up (dense back-to-back matmuls) ----
            with tc.tile_pool(name="psw", bufs=1, space="PSUM") as psw:
                wps = psw.tile([128, 512], dt.float32, tag="wps", name="wps")
                for i in range(24):
                    nc.tensor.matmul(wps[:], Wo_sb[0][:, 0:128],
                                     Wo_sb[0][:, 0:512],
                                     start=(i == 0), stop=(i == 23))
                wsb = oev.tile([128, 512], dt.float32, tag="wsb", name="wsb")
                nc.vector.tensor_copy(wsb[:], wps[:])
                nc.sync.dma_start(warm_out, wsb[:])

            # ---- phase 1: projections ----
            with tc.tile_pool(name="psp", bufs=1, space="PSUM") as psp:
                # K then V then Q (attention needs all of k/v but only the
                # leading q-chunk of qhT to start).
                for name, src, FT, bias_sb, dstT in (
                    ("k", kT, Fk_sb, bfk_sb, khT),
                    ("q", qT, Fq_sb, bfq_sb, qhT),
                ):
                    xt = [xin.tile([128, S], bf, tag="xin", name="xin") for _ in range(ECH)]
                    for e in range(ECH):
                        nc.sync.dma_start(xt[e][:], src[bass.ts(e, 128), :])
                    for pair in range(2):
                        for nn in range(NQ):  # N-chunks of 512
                            pp = psp.tile([128, 512], dt.float32, tag="pp",
                                          name="pp", bufs=3)
                            for e in range(ECH):
                                nc.tensor.matmul(
                                    pp[:],
                                    FT[e][:, bass.ts(pair, 128)],
                                    xt[e][:, bass.ts(nn, 512)],
                                    start=(e == 0), stop=(e == ECH - 1),
                                )
                            nc.vector.tensor_scalar(
                                dstT[pair][:, bass.ts(nn, 512)], pp[:],
                                bias_sb[:, bass.ds(pair, 1)], None, op0=OP.add,
                            )
                    if name == "k":
                        # V projection: natural [S, 4*65] layout w/ ones cols
                        vt = [xin.tile([128, S], bf, tag="xin", name="xin") for _ in range(ECH)]
                        for e in range(ECH):
                            nc.sync.dma_start(vt[e][:], vT[bass.ts(e, 128), :])
                        for sc in range(NK):
                            pv = psp.tile([128, 260], dt.float32, tag="pv",
                                          name="pv", bufs=2)
                            for e in range(ECH):
                                nc.tensor.matmul(
                                    pv[:], vt[e][:, bass.ts(sc, 128)], Fv_sb[e][:],
                                    start=(e == 0), stop=False,
                                )
                            nc.tensor.matmul(
                                pv[:], onesb_sb[:, 0:128], bfv_sb[:],
                                start=False, stop=True,
                            )
                            nc.vector.tensor_copy(vh_sb[sc][:], pv[:])

            # ---- phase 2: attention ----
            with tc.tile_pool(name="psa", bufs=1, space="PSUM") as psa:
                for qc in range(NQ):
                    outs = [psa.tile([65, 512], dt.float32, tag=f"out{h}",
                                     name=f"out{h}") for h in range(HPC)]
                    for kc in range(NK):
                        mt = maskp.tile([128, 512], bf, tag="mask", name="mask")
                        nc.sync.dma_start(
                            mt[:], maskT[bass.ts(kc, 128), bass.ts(qc, 512)])
                        escs = []
                        for pair in range(2):
                            sT = psa.tile([128, 1024], dt.float32, tag="sT",
                                          name="sT", bufs=2)
                            for half in range(2):
                                lo = half * 64
                                nc.tensor.matmul(
                                    sT[:, bass.ts(half, 512)],
                                    khT[pair][lo:lo + 64, bass.ts(kc, 128)],
                                    qhT[pair][lo:lo + 64, bass.ts(qc, 512)],
                                    start=True, stop=True,
                                )
                            esc = escp.tile([128, 1024], bf, tag="esc", name="esc")
                            nc.scalar.activation(esc[:], sT[:], AF.Exp)
                            for half in range(2):
                                nc.vector.tensor_tensor(
                                    esc[:, bass.ts(half, 512)],
                                    esc[:, bass.ts(half, 512)], mt[:], op=OP.mult)
                            escs.append(esc)
                        for h in range(HPC):
                            pair, half = h // 2, h % 2
                            nc.tensor.matmul(
                                outs[h][:],
                                vh_sb[kc][:, bass.ds(65 * h, 65)],
                                escs[pair][:, bass.ts(half, 512)],
                                start=(kc == 0), stop=(kc == NK - 1),
                            )
                    # normalize -> xT: per-head sums land on partitions
                    # {0,32,64,96} of sums128 (32-aligned engine access),
                    # one batched reciprocal, then selector-matmul broadcast.
                    avs = []
                    sums128 = avnp.tile([128, 512], dt.float32, tag="sums128",
                                        name="sums128")
                    nc.vector.memset(sums128[:], 1.0)
                    for h in range(HPC):
                        av = avnp.tile([64, 512], dt.float32, tag=f"av{h}",
                                       name=f"av{h}")
                        nc.scalar.copy(av[:], outs[h][0:64, :])
                        nc.vector.tensor_copy(sums128[32 * h:32 * h + 1, :],
                                              outs[h][64:65, :])
                        avs.append(av)
                    recip128 = avnp.tile([128, 512], dt.float32r, tag="recip128",
                                         name="recip128")
                    with nc.allow_low_precision(reason="softmax denominators"):
                        nc.vector.reciprocal(recip128[:], sums128[:])
                    for h in range(HPC):
                        pair, lo = h // 2, (h % 2) * 64
                        bc = psa.tile([64, 512], dt.float32, tag="sT", name="bc", bufs=2)
                        nc.tensor.matmul(bc[:], sel_sb[:, bass.ds(64 * h, 64)],
                                         recip128[:], start=True, stop=True)
                        nc.vector.tensor_tensor(
                            xT_sb[pair][lo:lo + 64, bass.ts(qc, 512)],
                            avs[h][0:64, :], bc[:], op=OP.mult)
                    # this q-chunk's slice of the output projection: fills the
                    # PE gap at the q-chunk boundary and overlaps DMA-out.
                    for eo in range(ECH):
                        po = psa.tile([128, 512], dt.float32, tag="sT",
                                      name="po", bufs=2)
                        for c in range(2):
                            nc.tensor.matmul(
                                po[:], Wo_sb[c][:, bass.ts(eo, 128)],
                                xT_sb[c][:, bass.ts(qc, 512)],
                                start=(c == 0), stop=(c == 1),
                            )
                        ot = oev.tile([128, 512], dt.float32, tag="ot",
                                      name="ot", bufs=4)
                        if eo % 2 == 0:
                            nc.vector.tensor_copy(ot[:], po[:])
                        else:
                            nc.scalar.copy(ot[:], po[:])
                        nc.sync.dma_start(
                            out_pT[bass.ts(eo, 128), bass.ts(qc, 512)], ot[:])


    nc.compile()
    _prog_cache["nc"] = nc
    return nc


def prep_inputs(q_matrix, k_matrix, v_matrix, mask, Wq, bq, Wk, bk, Wv, bv,
                Whq, bhq, Whk, bhk, Whv, bhv, Wo, bo):
    f32 = np.float32
    bf16 = ml_dtypes.bfloat16
    q_matrix = np.asarray(q_matrix, f32)
    k_matrix = np.asarray(k_matrix, f32)
    v_matrix = np.asarray(v_matrix, f32)
    mask = np.asarray(mask)
    sc = f32(1.0 / np.sqrt(HD))

    Wq, Wk, Wv = np.asarray(Wq, f32), np.asarray(Wk, f32), np.asarray(Wv, f32)
    Whq, Whk, Whv = np.asarray(Whq, f32), np.asarray(Whk, f32), np.asarray(Whv, f32)
    bq, bk, bv = np.asarray(bq, f32), np.asarray(bk, f32), np.asarray(bv, f32)
    bhq, bhk, bhv = np.asarray(bhq, f32), np.asarray(bhk, f32), np.asarray(bhv, f32)
    # Fx[h] = Wx @ Whx[h]: one BLAS call via tensordot -> [E(out), H, HD]
    FqH = (np.tensordot(Wq, Whq, axes=([1], [1])) * sc).astype(f32)
    FkH = np.tensordot(Wk, Whk, axes=([1], [1])).astype(f32)
    FvH = np.tensordot(Wv, Whv, axes=([1], [1])).astype(f32)
    bqH = ((np.einsum("e,hed->hd", bq, Whq) + bhq) * sc).astype(f32)
    bkH = (np.einsum("e,hed->hd", bk, Whk) + bhk).astype(f32)
    bvH = (np.einsum("e,hed->hd", bv, Whv) + bhv).astype(f32)
    WoM = np.asarray(Wo, f32)

    onesb = np.ones((1, 128), f32).astype(bf16)
    sel = np.zeros((128, 256), f32)
    for h in range(4):
        sel[32 * h, 64 * h:64 * (h + 1)] = 1.0
    in_maps = []
    for core in range(N_CORES):
        b, g = core // 4, core % 4
        hs = [4 * g + j for j in range(4)]
        Fq_c = np.ascontiguousarray(FqH[:, hs, :].reshape(E, 256)).astype(bf16)
        Fk_c = np.ascontiguousarray(FkH[:, hs, :].reshape(E, 256)).astype(bf16)
        Fv_c = np.zeros((E, 260), f32)
        bfv_c = np.zeros((1, 260), f32)
        for j, h in enumerate(hs):
            Fv_c[:, 65 * j:65 * j + 64] = FvH[:, h, :]
            bfv_c[0, 65 * j:65 * j + 64] = bvH[h]
            bfv_c[0, 65 * j + 64] = 1.0
        bfq_c = np.stack([np.concatenate([bqH[hs[2 * p]], bqH[hs[2 * p + 1]]])
                          for p in range(2)], axis=1)                # [128, 2]
        bfk_c = np.stack([np.concatenate([bkH[hs[2 * p]], bkH[hs[2 * p + 1]]])
                          for p in range(2)], axis=1)
        in_maps.append(dict(
            qT=np.ascontiguousarray(q_matrix[b].T).astype(bf16),
            kT=np.ascontiguousarray(k_matrix[b].T).astype(bf16),
            vT=np.ascontiguousarray(v_matrix[b].T).astype(bf16),
            maskT=np.ascontiguousarray(
                (~mask[b].T).astype(np.float32)).astype(bf16),
            Fq=Fq_c, Fk=Fk_c, Fv=Fv_c.astype(bf16),
            bfq=bfq_c, bfk=bfk_c, bfv=bfv_c.astype(bf16),
            Wo=np.ascontiguousarray(WoM[256 * g:256 * (g + 1), :]).astype(bf16),
            onesb=onesb, sel=sel,
        ))
    return in_maps


def unshard(results, bo):
    bo = np.asarray(bo, np.float32)
    out = np.empty((B, S, E), np.float32)
    for b in range(B):
        acc = results[4 * b]["out_pT"].astype(np.float32).copy()
        for g in range(1, 4):
            acc += results[4 * b + g]["out_pT"]
        out[b] = acc.T + bo
    return out


def kernel(**inputs):
    from concourse.bass_utils import run_bass_kernel_spmd
    nc = build_program()
    in_maps = prep_inputs(**inputs)
    res = run_bass_kernel_spmd(nc, in_maps, list(range(N_CORES)))
    return unshard(res.results, inputs["bo"])


# revision 14
# speedup vs baseline: 1.0026x; 1.0026x over previous
"""MultiHeadAttention Trainium2 kernel (8 NeuronCores).

Problem: B=2, S=2048, E=1024, H=16, HD=64.
  qg = q @ Wq + bq ; qh[h] = qg @ Whq[h] + bhq[h]   (same for k, v)
  scores = qh @ kh^T / sqrt(HD), masked (-inf where mask), softmax
  out = concat_h(softmax @ vh) @ Wo + bo

Sharding: core c = 4*b + g handles batch b, heads 4g..4g+3 (data parallel on
B, tensor parallel on H). The global+per-head projections are folded on the
host into per-head fused weights Fq[h] = Wq @ Whq[h] (etc.), so each core
runs one [E, HD] projection per head. The output projection is row-sharded:
each core computes Wo[256g:256g+256]^T @ x^T and the host sums partials.

Data path is bf16 (matches the PE's full-rate mode + FWL weight loads +
DVE 2x mode); all matmul accumulation is fp32 in PSUM, and the softmax
normalization (reciprocal of denominators) runs in f32r.

On-chip layout (per core):
  qT/kT/vT [E=1024, S=2048] bf16  (host pre-transposed)
  qhT/khT: two [128, 2048] tiles, heads (2p, 2p+1) stacked on partition
    halves -> QK matmuls for adjacent heads land on disjoint PE row groups.
  vh: 16 tiles [128 (k-chunk), 4*65]; per head 64 value cols + a fused
    ones column (projection with zero weights, bias 1) so the AV matmul
    also produces softmax denominators.
  scores kept transposed [k, q]: softmax reduction becomes a matmul
    contraction; the probability tile is directly the AV moving operand.
  Per (q-chunk, k-chunk): one [128, 1024] PSUM scores tile per head-pair,
  one exp over the pair, per-head mask multiply (keep-mask, post-exp),
  AV accumulate into per-head [65, 512] PSUM banks.
"""
import ml_dtypes
import numpy as np
from contextlib import ExitStack

import concourse.bass as bass
import concourse.mybir as mybir
import concourse.tile as tile
from concourse import bacc

dt = mybir.dt
AF = mybir.ActivationFunctionType
OP = mybir.AluOpType

B, S, E, H = 2, 2048, 1024, 16
HD = E // H          # 64
HPC = H // 4         # heads per core = 4
N_CORES = 8
ECH = E // 128       # 8 e-chunks
NQ = S // 512        # 4 q chunks
NK = S // 128        # 16 k chunks

_prog_cache = {}


def build_program():
    if "nc" in _prog_cache:
        return _prog_cache["nc"]
    nc = bacc.Bacc("TRN2", target_bir_lowering=False, debug=False,
                   num_devices=N_CORES)

    bf = dt.bfloat16
    qT = nc.dram_tensor("qT", [E, S], bf, kind="ExternalInput").ap()
    kT = nc.dram_tensor("kT", [E, S], bf, kind="ExternalInput").ap()
    vT = nc.dram_tensor("vT", [E, S], bf, kind="ExternalInput").ap()
    maskT = nc.dram_tensor("maskT", [S, S], bf, kind="ExternalInput").ap()
    Fq = nc.dram_tensor("Fq", [E, 256], bf, kind="ExternalInput").ap()
    Fk = nc.dram_tensor("Fk", [E, 256], bf, kind="ExternalInput").ap()
    Fv = nc.dram_tensor("Fv", [E, 260], bf, kind="ExternalInput").ap()
    bfq = nc.dram_tensor("bfq", [128, 2], dt.float32, kind="ExternalInput").ap()
    bfk = nc.dram_tensor("bfk", [128, 2], dt.float32, kind="ExternalInput").ap()
    bfv = nc.dram_tensor("bfv", [1, 260], bf, kind="ExternalInput").ap()
    Wo = nc.dram_tensor("Wo", [256, 1024], bf, kind="ExternalInput").ap()
    onesb = nc.dram_tensor("onesb", [1, 128], bf, kind="ExternalInput").ap()
    # head-selector for denominator broadcast: row h ones in cols 64h..64h+63
    sel = nc.dram_tensor("sel", [128, 256], dt.float32r, kind="ExternalInput").ap()
    out_pT = nc.dram_tensor("out_pT", [E, S], dt.float32, kind="ExternalOutput").ap()
    warm_out = nc.dram_tensor("warm_out", [128, 512], dt.float32, kind="ExternalOutput").ap()

    with tile.TileContext(nc) as tc:
        with ExitStack() as ctx:
            wc = ctx.enter_context(tc.tile_pool(name="wc", bufs=1))
            xin = ctx.enter_context(tc.tile_pool(name="xin", bufs=12))
            qk = ctx.enter_context(tc.tile_pool(name="qk", bufs=1))
            vhp = ctx.enter_context(tc.tile_pool(name="vhp", bufs=1))
            xTp = ctx.enter_context(tc.tile_pool(name="xTp", bufs=1))
            maskp = ctx.enter_context(tc.tile_pool(name="maskp", bufs=6))
            escp = ctx.enter_context(tc.tile_pool(name="escp", bufs=8))
            avnp = ctx.enter_context(tc.tile_pool(name="avnp", bufs=2))
            oev = ctx.enter_context(tc.tile_pool(name="oev", bufs=2))

            # ---- constants ----
            Fq_sb = [wc.tile([128, 256], bf, tag=f"Fq{e}", name=f"Fq{e}") for e in range(ECH)]
            Fk_sb = [wc.tile([128, 256], bf, tag=f"Fk{e}", name=f"Fk{e}") for e in range(ECH)]
            Fv_sb = [wc.tile([128, 260], bf, tag=f"Fv{e}", name=f"Fv{e}") for e in range(ECH)]
            for e in range(ECH):
                nc.sync.dma_start(Fq_sb[e][:], Fq[bass.ts(e, 128), :])
                nc.sync.dma_start(Fk_sb[e][:], Fk[bass.ts(e, 128), :])
                nc.sync.dma_start(Fv_sb[e][:], Fv[bass.ts(e, 128), :])
            Wo_sb = [wc.tile([128, 1024], bf, tag=f"Wo{c}", name=f"Wo{c}") for c in range(2)]
            for c in range(2):
                nc.sync.dma_start(Wo_sb[c][:], Wo[bass.ts(c, 128), :])
            bfq_sb = wc.tile([128, 2], dt.float32, tag="bfq")
            bfk_sb = wc.tile([128, 2], dt.float32, tag="bfk")
            bfv_sb = wc.tile([1, 260], bf, tag="bfv")
            onesb_sb = wc.tile([1, 128], bf, tag="onesb")
            sel_sb = wc.tile([128, 256], dt.float32r, tag="sel")
            nc.sync.dma_start(bfq_sb[:], bfq)
            nc.sync.dma_start(bfk_sb[:], bfk)
            nc.sync.dma_start(bfv_sb[:], bfv)
            nc.sync.dma_start(onesb_sb[:], onesb)
            nc.sync.dma_start(sel_sb[:], sel)

            qhT = [qk.tile([128, S], bf, tag=f"qhT{p}", name=f"qhT{p}") for p in range(2)]
            khT = [qk.tile([128, S], bf, tag=f"khT{p}", name=f"khT{p}") for p in range(2)]
            vh_sb = [vhp.tile([128, 4 * 65], bf, tag=f"vh{sc}", name=f"vh{sc}") for sc in range(NK)]
            xT_sb = [xTp.tile([128, S], bf, tag=f"xT{c}", name=f"xT{c}") for c in range(2)]

            # ---- phase 0: PE warm-up (dense back-to-back matmuls) ----
            with tc.tile_pool(name="psw", bufs=1, space="PSUM") as psw:
                wps = psw.tile([128, 512], dt.float32, tag="wps", name="wps")
                for i in range(24):
                    nc.tensor.matmul(wps[:], Wo_sb[0][:, 0:128],
                                     Wo_sb[0][:, 0:512],
                                     start=(i == 0), stop=(i == 23))
                wsb = oev.tile([128, 512], dt.float32, tag="wsb", name="wsb")
                nc.vector.tensor_copy(wsb[:], wps[:])
                nc.sync.dma_start(warm_out, wsb[:])

            # ---- phase 1: projections ----
            with tc.tile_pool(name="psp", bufs=1, space="PSUM") as psp:
                # K then V then Q (attention needs all of k/v but only the
                # leading q-chunk of qhT to start).
                for name, src, FT, bias_sb, dstT in (
                    ("k", kT, Fk_sb, bfk_sb, khT),
                    ("q", qT, Fq_sb, bfq_sb, qhT),
                ):
                    xt = [xin.tile([128, S], bf, tag="xin", name="xin") for _ in range(ECH)]
                    for e in range(ECH):
                        nc.sync.dma_start(xt[e][:], src[bass.ts(e, 128), :])
                    for nn in range(NQ):  # N-chunks of 512
                        for pair in range(2):
                            pp = psp.tile([128, 512], dt.float32, tag="pp",
                                          name="pp", bufs=3)
                            for e in range(ECH):
                                nc.tensor.matmul(
                                    pp[:],
                                    FT[e][:, bass.ts(pair, 128)],
                                    xt[e][:, bass.ts(nn, 512)],
                                    start=(e == 0), stop=(e == ECH - 1),
                                )
                            nc.vector.tensor_scalar(
                                dstT[pair][:, bass.ts(nn, 512)], pp[:],
                                bias_sb[:, bass.ds(pair, 1)], None, op0=OP.add,
                            )
                    if name == "k":
                        # V projection: natural [S, 4*65] layout w/ ones cols
                        vt = [xin.tile([128, S], bf, tag="xin", name="xin") for _ in range(ECH)]
                        for e in range(ECH):
                            nc.sync.dma_start(vt[e][:], vT[bass.ts(e, 128), :])
                        for sc in range(NK):
                            pv = psp.tile([128, 260], dt.float32, tag="pv",
                                          name="pv", bufs=2)
                            for e in range(ECH):
                                nc.tensor.matmul(
                                    pv[:], vt[e][:, bass.ts(sc, 128)], Fv_sb[e][:],
                                    start=(e == 0), stop=False,
                                )
                            nc.tensor.matmul(
                                pv[:], onesb_sb[:, 0:128], bfv_sb[:],
                                start=False, stop=True,
                            )
                            nc.vector.tensor_copy(vh_sb[sc][:], pv[:])

            # ---- phase 2: attention ----
            with tc.tile_pool(name="psa", bufs=1, space="PSUM") as psa:
                def normalize_and_oproj(qc, outs):
                    # normalize -> xT: per-head sums land on partitions
                    # {0,32,64,96} of sums128 (32-aligned engine access),
                    # one batched reciprocal, then selector-matmul broadcast.
                    avs = []
                    sums128 = avnp.tile([128, 512], dt.float32, tag="sums128",
                                        name="sums128")
                    nc.vector.memset(sums128[:], 1.0)
                    for h in range(HPC):
                        av = avnp.tile([64, 512], dt.float32, tag=f"av{h}",
                                       name=f"av{h}")
                        nc.vector.tensor_copy(av[:], outs[h][0:64, :])
                        nc.scalar.copy(sums128[32 * h:32 * h + 1, :],
                                       outs[h][64:65, :])
                        avs.append(av)
                    recip128 = avnp.tile([128, 512], dt.float32r, tag="recip128",
                                         name="recip128")
                    with nc.allow_low_precision(reason="softmax denominators"):
                        nc.vector.reciprocal(recip128[:], sums128[:])
                    for h in range(HPC):
                        pair, lo = h // 2, (h % 2) * 64
                        bc = psa.tile([64, 512], dt.float32, tag="sT", name="bc", bufs=2)
                        nc.tensor.matmul(bc[:], sel_sb[:, bass.ds(64 * h, 64)],
                                         recip128[:], start=True, stop=True)
                        nc.vector.tensor_tensor(
                            xT_sb[pair][lo:lo + 64, bass.ts(qc, 512)],
                            avs[h][0:64, :], bc[:], op=OP.mult)
                    # this q-chunk's slice of the output projection.
                    for eo in range(ECH):
                        po = psa.tile([128, 512], dt.float32, tag="sT",
                                      name="po", bufs=2)
                        for c in range(2):
                            nc.tensor.matmul(
                                po[:], Wo_sb[c][:, bass.ts(eo, 128)],
                                xT_sb[c][:, bass.ts(qc, 512)],
                                start=(c == 0), stop=(c == 1),
                            )
                        ot = oev.tile([128, 512], dt.float32, tag="ot",
                                      name="ot", bufs=4)
                        if eo % 2 == 0:
                            nc.vector.tensor_copy(ot[:], po[:])
                        else:
                            nc.scalar.copy(ot[:], po[:])
                        nc.sync.dma_start(
                            out_pT[bass.ts(eo, 128), bass.ts(qc, 512)], ot[:])

                prev = None  # (qc, outs) awaiting normalize
                for qc in range(NQ):
                    outs = [psa.tile([65, 512], dt.float32, tag=f"out{h}",
                                     name=f"out{h}") for h in range(HPC)]
                    for kc in range(NK):
                        mt = maskp.tile([128, 512], bf, tag="mask", name="mask")
                        nc.sync.dma_start(
                            mt[:], maskT[bass.ts(kc, 128), bass.ts(qc, 512)])
                        escs = []
                        for pair in range(2):
                            sT = psa.tile([128, 1024], dt.float32, tag="sT",
                                          name="sT", bufs=2)
                            for half in range(2):
                                lo = half * 64
                                nc.tensor.matmul(
                                    sT[:, bass.ts(half, 512)],
                                    khT[pair][lo:lo + 64, bass.ts(kc, 128)],
                                    qhT[pair][lo:lo + 64, bass.ts(qc, 512)],
                                    start=True, stop=True,
                                )
                            esc = escp.tile([128, 1024], bf, tag="esc", name="esc")
                            nc.scalar.activation(esc[:], sT[:], AF.Exp)
                            for half in range(2):
                                nc.vector.tensor_tensor(
                                    esc[:, bass.ts(half, 512)],
                                    esc[:, bass.ts(half, 512)], mt[:], op=OP.mult)
                            escs.append(esc)
                        for h in range(HPC):
                            pair, half = h // 2, h % 2
                            nc.tensor.matmul(
                                outs[h][:],
                                vh_sb[kc][:, bass.ds(65 * h, 65)],
                                escs[pair][:, bass.ts(half, 512)],
                                start=(kc == 0), stop=(kc == NK - 1),
                            )
                        if kc == 0 and prev is not None:
                            # emit the previous q-chunk's normalize/oproj here
                            # so this chunk's exps win the ACT queue at the
                            # boundary; the scheduler overlaps the rest.
                            normalize_and_oproj(*prev)
                            prev = None
                    prev = (qc, outs)
                normalize_and_oproj(*prev)


    nc.compile()
    _prog_cache["nc"] = nc
    return nc


def prep_inputs(q_matrix, k_matrix, v_matrix, mask, Wq, bq, Wk, bk, Wv, bv,
                Whq, bhq, Whk, bhk, Whv, bhv, Wo, bo):
    f32 = np.float32
    bf16 = ml_dtypes.bfloat16
    q_matrix = np.asarray(q_matrix, f32)
    k_matrix = np.asarray(k_matrix, f32)
    v_matrix = np.asarray(v_matrix, f32)
    mask = np.asarray(mask)
    sc = f32(1.0 / np.sqrt(HD))

    Wq, Wk, Wv = np.asarray(Wq, f32), np.asarray(Wk, f32), np.asarray(Wv, f32)
    Whq, Whk, Whv = np.asarray(Whq, f32), np.asarray(Whk, f32), np.asarray(Whv, f32)
    bq, bk, bv = np.asarray(bq, f32), np.asarray(bk, f32), np.asarray(bv, f32)
    bhq, bhk, bhv = np.asarray(bhq, f32), np.asarray(bhk, f32), np.asarray(bhv, f32)
    # Fx[h] = Wx @ Whx[h]: one BLAS call via tensordot -> [E(out), H, HD]
    FqH = (np.tensordot(Wq, Whq, axes=([1], [1])) * sc).astype(f32)
    FkH = np.tensordot(Wk, Whk, axes=([1], [1])).astype(f32)
    FvH = np.tensordot(Wv, Whv, axes=([1], [1])).astype(f32)
    bqH = ((np.einsum("e,hed->hd", bq, Whq) + bhq) * sc).astype(f32)
    bkH = (np.einsum("e,hed->hd", bk, Whk) + bhk).astype(f32)
    bvH = (np.einsum("e,hed->hd", bv, Whv) + bhv).astype(f32)
    WoM = np.asarray(Wo, f32)

    onesb = np.ones((1, 128), f32).astype(bf16)
    sel = np.zeros((128, 256), f32)
    for h in range(4):
        sel[32 * h, 64 * h:64 * (h + 1)] = 1.0
    in_maps = []
    for core in range(N_CORES):
        b, g = core // 4, core % 4
        hs = [4 * g + j for j in range(4)]
        Fq_c = np.ascontiguousarray(FqH[:, hs, :].reshape(E, 256)).astype(bf16)
        Fk_c = np.ascontiguousarray(FkH[:, hs, :].reshape(E, 256)).astype(bf16)
        Fv_c = np.zeros((E, 260), f32)
        bfv_c = np.zeros((1, 260), f32)
        for j, h in enumerate(hs):
            Fv_c[:, 65 * j:65 * j + 64] = FvH[:, h, :]
            bfv_c[0, 65 * j:65 * j + 64] = bvH[h]
            bfv_c[0, 65 * j + 64] = 1.0
        bfq_c = np.stack([np.concatenate([bqH[hs[2 * p]], bqH[hs[2 * p + 1]]])
                          for p in range(2)], axis=1)                # [128, 2]
        bfk_c = np.stack([np.concatenate([bkH[hs[2 * p]], bkH[hs[2 * p + 1]]])
                          for p in range(2)], axis=1)
        in_maps.append(dict(
            qT=np.ascontiguousarray(q_matrix[b].T).astype(bf16),
            kT=np.ascontiguousarray(k_matrix[b].T).astype(bf16),
            vT=np.ascontiguousarray(v_matrix[b].T).astype(bf16),
            maskT=np.ascontiguousarray(
                (~mask[b].T).astype(np.float32)).astype(bf16),
            Fq=Fq_c, Fk=Fk_c, Fv=Fv_c.astype(bf16),
            bfq=bfq_c, bfk=bfk_c, bfv=bfv_c.astype(bf16),
            Wo=np.ascontiguousarray(WoM[256 * g:256 * (g + 1), :]).astype(bf16),
            onesb=onesb, sel=sel,
        ))
    return in_maps


def unshard(results, bo):
    bo = np.asarray(bo, np.float32)
    out = np.empty((B, S, E), np.float32)
    for b in range(B):
        acc = results[4 * b]["out_pT"].astype(np.float32).copy()
        for g in range(1, 4):
            acc += results[4 * b + g]["out_pT"]
        out[b] = acc.T + bo
    return out


def kernel(**inputs):
    from concourse.bass_utils import run_bass_kernel_spmd
    nc = build_program()
    in_maps = prep_inputs(**inputs)
    res = run_bass_kernel_spmd(nc, in_maps, list(range(N_CORES)))
    return unshard(res.results, inputs["bo"])


# revision 16
# speedup vs baseline: 1.0375x; 1.0348x over previous
"""MultiHeadAttention Trainium2 kernel (8 NeuronCores).

Problem: B=2, S=2048, E=1024, H=16, HD=64.
  qg = q @ Wq + bq ; qh[h] = qg @ Whq[h] + bhq[h]   (same for k, v)
  scores = qh @ kh^T / sqrt(HD), masked (-inf where mask), softmax
  out = concat_h(softmax @ vh) @ Wo + bo

Sharding: core c = 4*b + g handles batch b, heads 4g..4g+3 (data parallel on
B, tensor parallel on H). The global+per-head projections are folded on the
host into per-head fused weights Fq[h] = Wq @ Whq[h] (etc.), so each core
runs one [E, HD] projection per head. The output projection is row-sharded:
each core computes Wo[256g:256g+256]^T @ x^T and the host sums partials.

Data path is bf16 (matches the PE's full-rate mode + FWL weight loads +
DVE 2x mode); all matmul accumulation is fp32 in PSUM, and the softmax
normalization (reciprocal of denominators) runs in f32r.

On-chip layout (per core):
  qT/kT/vT [E=1024, S=2048] bf16  (host pre-transposed)
  qhT/khT: two [128, 2048] tiles, heads (2p, 2p+1) stacked on partition
    halves -> QK matmuls for adjacent heads land on disjoint PE row groups.
  vh: 16 tiles [128 (k-chunk), 4*65]; per head 64 value cols + a fused
    ones column (projection with zero weights, bias 1) so the AV matmul
    also produces softmax denominators.
  scores kept transposed [k, q]: softmax reduction becomes a matmul
    contraction; the probability tile is directly the AV moving operand.
  Per (q-chunk, k-chunk): one [128, 1024] PSUM scores tile per head-pair,
  one exp over the pair, per-head mask multiply (keep-mask, post-exp),
  AV accumulate into per-head [65, 512] PSUM banks.
"""
import ml_dtypes
import numpy as np
from contextlib import ExitStack

import concourse.bass as bass
import concourse.mybir as mybir
import concourse.tile as tile
from concourse import bacc

dt = mybir.dt
AF = mybir.ActivationFunctionType
OP = mybir.AluOpType

B, S, E, H = 2, 2048, 1024, 16
HD = E // H          # 64
HPC = H // 4         # heads per core = 4
N_CORES = 8
ECH = E // 128       # 8 e-chunks
NQ = S // 512        # 4 q chunks
NK = S // 128        # 16 k chunks

_prog_cache = {}


def build_program():
    if "nc" in _prog_cache:
        return _prog_cache["nc"]
    nc = bacc.Bacc("TRN2", target_bir_lowering=False, debug=False,
                   num_devices=N_CORES)

    bf = dt.bfloat16
    qT = nc.dram_tensor("qT", [E, S], bf, kind="ExternalInput").ap()
    kT = nc.dram_tensor("kT", [E, S], bf, kind="ExternalInput").ap()
    vT = nc.dram_tensor("vT", [E, S], bf, kind="ExternalInput").ap()
    maskT = nc.dram_tensor("maskT", [S, S], bf, kind="ExternalInput").ap()
    Fq = nc.dram_tensor("Fq", [E, 256], bf, kind="ExternalInput").ap()
    Fk = nc.dram_tensor("Fk", [E, 256], bf, kind="ExternalInput").ap()
    Fv = nc.dram_tensor("Fv", [E, 260], bf, kind="ExternalInput").ap()
    bfq = nc.dram_tensor("bfq", [128, 2], dt.float32, kind="ExternalInput").ap()
    bfk = nc.dram_tensor("bfk", [128, 2], dt.float32, kind="ExternalInput").ap()
    bfv = nc.dram_tensor("bfv", [1, 260], bf, kind="ExternalInput").ap()
    Wo = nc.dram_tensor("Wo", [256, 1024], bf, kind="ExternalInput").ap()
    onesb = nc.dram_tensor("onesb", [1, 128], bf, kind="ExternalInput").ap()
    # head-selector for denominator broadcast: row h ones in cols 64h..64h+63
    sel = nc.dram_tensor("sel", [128, 256], dt.float32r, kind="ExternalInput").ap()
    out_pT = nc.dram_tensor("out_pT", [E, S], dt.float32, kind="ExternalOutput").ap()
    warm_out = nc.dram_tensor("warm_out", [128, 512], dt.float32, kind="ExternalOutput").ap()

    with tile.TileContext(nc) as tc:
        with ExitStack() as ctx:
            wc = ctx.enter_context(tc.tile_pool(name="wc", bufs=1))
            xin = ctx.enter_context(tc.tile_pool(name="xin", bufs=12))
            qk = ctx.enter_context(tc.tile_pool(name="qk", bufs=1))
            vhp = ctx.enter_context(tc.tile_pool(name="vhp", bufs=1))
            xTp = ctx.enter_context(tc.tile_pool(name="xTp", bufs=1))
            maskp = ctx.enter_context(tc.tile_pool(name="maskp", bufs=6))
            escp = ctx.enter_context(tc.tile_pool(name="escp", bufs=8))
            avnp = ctx.enter_context(tc.tile_pool(name="avnp", bufs=2))
            oev = ctx.enter_context(tc.tile_pool(name="oev", bufs=2))

            # ---- constants (Wo first: feeds the PE warm-up) ----
            Wo_sb = [wc.tile([128, 1024], bf, tag=f"Wo{c}", name=f"Wo{c}") for c in range(2)]
            for c in range(2):
                nc.sync.dma_start(Wo_sb[c][:], Wo[bass.ts(c, 128), :])
            Fq_sb = [wc.tile([128, 256], bf, tag=f"Fq{e}", name=f"Fq{e}") for e in range(ECH)]
            Fk_sb = [wc.tile([128, 256], bf, tag=f"Fk{e}", name=f"Fk{e}") for e in range(ECH)]
            Fv_sb = [wc.tile([128, 260], bf, tag=f"Fv{e}", name=f"Fv{e}") for e in range(ECH)]
            for e in range(ECH):
                nc.sync.dma_start(Fk_sb[e][:], Fk[bass.ts(e, 128), :])
                nc.sync.dma_start(Fv_sb[e][:], Fv[bass.ts(e, 128), :])
                nc.sync.dma_start(Fq_sb[e][:], Fq[bass.ts(e, 128), :])
            bfq_sb = wc.tile([128, 2], dt.float32, tag="bfq")
            bfk_sb = wc.tile([128, 2], dt.float32, tag="bfk")
            bfv_sb = wc.tile([1, 260], bf, tag="bfv")
            onesb_sb = wc.tile([1, 128], bf, tag="onesb")
            sel_sb = wc.tile([128, 256], dt.float32r, tag="sel")
            nc.sync.dma_start(bfq_sb[:], bfq)
            nc.sync.dma_start(bfk_sb[:], bfk)
            nc.sync.dma_start(bfv_sb[:], bfv)
            nc.sync.dma_start(onesb_sb[:], onesb)
            nc.sync.dma_start(sel_sb[:], sel)

            qhT = [qk.tile([128, S], bf, tag=f"qhT{p}", name=f"qhT{p}") for p in range(2)]
            khT = [qk.tile([128, S], bf, tag=f"khT{p}", name=f"khT{p}") for p in range(2)]
            vh_sb = [vhp.tile([128, 4 * 65], bf, tag=f"vh{sc}", name=f"vh{sc}") for sc in range(NK)]
            xT_sb = [xTp.tile([128, S], bf, tag=f"xT{c}", name=f"xT{c}") for c in range(2)]

            # ---- phase 0: PE warm-up (dense back-to-back matmuls) ----
            with tc.tile_pool(name="psw", bufs=1, space="PSUM") as psw:
                wps = psw.tile([128, 512], dt.float32, tag="wps", name="wps")
                for i in range(24):
                    nc.tensor.matmul(wps[:], Wo_sb[0][:, 0:128],
                                     Wo_sb[0][:, 0:512],
                                     start=(i == 0), stop=(i == 23))
                wsb = oev.tile([128, 512], dt.float32, tag="wsb", name="wsb")
                nc.vector.tensor_copy(wsb[:], wps[:])
                nc.sync.dma_start(warm_out, wsb[:])

            # ---- phase 1: K and V projections (Q is folded into the
            # attention loop, using its PSUM slots and ACT slack) ----
            qt = [xin.tile([128, S], bf, tag=f"qt{e}", name=f"qt{e}", bufs=1)
                  for e in range(ECH)]
            with tc.tile_pool(name="psp", bufs=1, space="PSUM") as psp:
                xt = [xin.tile([128, S], bf, tag="xin", name="xin") for _ in range(ECH)]
                for e in range(ECH):
                    nc.sync.dma_start(xt[e][:], kT[bass.ts(e, 128), :])
                for nn in range(NQ):  # N-chunks of 512
                    for pair in range(2):
                        pp = psp.tile([128, 512], dt.float32, tag="pp",
                                      name="pp", bufs=3)
                        for e in range(ECH):
                            nc.tensor.matmul(
                                pp[:],
                                Fk_sb[e][:, bass.ts(pair, 128)],
                                xt[e][:, bass.ts(nn, 512)],
                                start=(e == 0), stop=(e == ECH - 1),
                            )
                        nc.vector.tensor_scalar(
                            khT[pair][:, bass.ts(nn, 512)], pp[:],
                            bfk_sb[:, bass.ds(pair, 1)], None, op0=OP.add,
                        )
                # V projection: natural [S, 4*65] layout w/ ones cols
                vt = [xin.tile([128, S], bf, tag="xin", name="xin") for _ in range(ECH)]
                for e in range(ECH):
                    nc.sync.dma_start(vt[e][:], vT[bass.ts(e, 128), :])
                for e in range(ECH):
                    nc.sync.dma_start(qt[e][:], qT[bass.ts(e, 128), :])
                for sc in range(NK):
                    pv = psp.tile([128, 260], dt.float32, tag="pv",
                                  name="pv", bufs=2)
                    for e in range(ECH):
                        nc.tensor.matmul(
                            pv[:], vt[e][:, bass.ts(sc, 128)], Fv_sb[e][:],
                            start=(e == 0), stop=False,
                        )
                    nc.tensor.matmul(
                        pv[:], onesb_sb[:, 0:128], bfv_sb[:],
                        start=False, stop=True,
                    )
                    nc.vector.tensor_copy(vh_sb[sc][:], pv[:])

            # ---- phase 2: attention ----
            with tc.tile_pool(name="psa", bufs=1, space="PSUM") as psa:
                sums128 = avnp.tile([128, 512], dt.float32, tag="sums128",
                                    name="sums128", bufs=1)
                nc.vector.memset(sums128[:], 1.0)
                recip128 = avnp.tile([128, 512], dt.float32r, tag="recip128",
                                     name="recip128", bufs=1)

                def normalize_and_oproj(qc, outs):
                    # normalize -> xT: per-head sums land on partitions
                    # {0,32,64,96} of sums128 (32-aligned engine access),
                    # one batched reciprocal, then selector-matmul broadcast.
                    avs = []
                    for h in range(HPC):
                        av = avnp.tile([64, 512], dt.float32, tag=f"av{h}",
                                       name=f"av{h}")
                        if h % 2 == 0:
                            nc.vector.tensor_copy(av[:], outs[h][0:64, :])
                        else:
                            nc.scalar.copy(av[:], outs[h][0:64, :])
                        nc.scalar.copy(sums128[32 * h:32 * h + 1, :],
                                       outs[h][64:65, :])
                        avs.append(av)
                    with nc.allow_low_precision(reason="softmax denominators"):
                        nc.vector.reciprocal(recip128[:], sums128[:])
                    for h in range(HPC):
                        pair, lo = h // 2, (h % 2) * 64
                        bc = psa.tile([64, 512], dt.float32, tag="sT", name="bc", bufs=2)
                        nc.tensor.matmul(bc[:], sel_sb[:, bass.ds(64 * h, 64)],
                                         recip128[:], start=True, stop=True)
                        nc.vector.tensor_tensor(
                            xT_sb[pair][lo:lo + 64, bass.ts(qc, 512)],
                            avs[h][0:64, :], bc[:], op=OP.mult)
                    # this q-chunk's slice of the output projection.
                    for eo in range(ECH):
                        po = psa.tile([128, 512], dt.float32, tag="sT",
                                      name="po", bufs=2)
                        for c in range(2):
                            nc.tensor.matmul(
                                po[:], Wo_sb[c][:, bass.ts(eo, 128)],
                                xT_sb[c][:, bass.ts(qc, 512)],
                                start=(c == 0), stop=(c == 1),
                            )
                        ot = oev.tile([128, 512], dt.float32, tag="ot",
                                      name="ot", bufs=4)
                        if eo % 2 == 0:
                            nc.vector.tensor_copy(ot[:], po[:])
                        else:
                            nc.scalar.copy(ot[:], po[:])
                        nc.sync.dma_start(
                            out_pT[bass.ts(eo, 128), bass.ts(qc, 512)], ot[:])

                prev = None  # (qc, outs) awaiting normalize
                for qc in range(NQ):
                    # q-projection for this q-chunk (2 pairs), evicted on ACT
                    for pair in range(2):
                        pq = psa.tile([128, 512], dt.float32, tag="sT",
                                      name="pq", bufs=2)
                        for e in range(ECH):
                            nc.tensor.matmul(
                                pq[:],
                                Fq_sb[e][:, bass.ts(pair, 128)],
                                qt[e][:, bass.ts(qc, 512)],
                                start=(e == 0), stop=(e == ECH - 1),
                            )
                        nc.scalar.activation(
                            qhT[pair][:, bass.ts(qc, 512)], pq[:], AF.Identity,
                            bias=bfq_sb[:, bass.ds(pair, 1)])
                    outs = [psa.tile([65, 512], dt.float32, tag=f"out{h}",
                                     name=f"out{h}") for h in range(HPC)]
                    for kc in range(NK):
                        mt = maskp.tile([128, 512], bf, tag="mask", name="mask")
                        nc.sync.dma_start(
                            mt[:], maskT[bass.ts(kc, 128), bass.ts(qc, 512)])
                        escs = []
                        for pair in range(2):
                            sT = psa.tile([128, 1024], dt.float32, tag="sT",
                                          name="sT", bufs=2)
                            for half in range(2):
                                lo = half * 64
                                nc.tensor.matmul(
                                    sT[:, bass.ts(half, 512)],
                                    khT[pair][lo:lo + 64, bass.ts(kc, 128)],
                                    qhT[pair][lo:lo + 64, bass.ts(qc, 512)],
                                    start=True, stop=True,
                                )
                            esc = escp.tile([128, 1024], bf, tag="esc", name="esc")
                            nc.scalar.activation(esc[:], sT[:], AF.Exp)
                            for half in range(2):
                                nc.vector.tensor_tensor(
                                    esc[:, bass.ts(half, 512)],
                                    esc[:, bass.ts(half, 512)], mt[:], op=OP.mult)
                            escs.append(esc)
                        for h in range(HPC):
                            pair, half = h // 2, h % 2
                            nc.tensor.matmul(
                                outs[h][:],
                                vh_sb[kc][:, bass.ds(65 * h, 65)],
                                escs[pair][:, bass.ts(half, 512)],
                                start=(kc == 0), stop=(kc == NK - 1),
                            )
                        if kc == 0 and prev is not None:
                            # emit the previous q-chunk's normalize/oproj here
                            # so this chunk's exps win the ACT queue at the
                            # boundary; the scheduler overlaps the rest.
                            normalize_and_oproj(*prev)
                            prev = None
                    prev = (qc, outs)
                normalize_and_oproj(*prev)


    nc.compile()
    _prog_cache["nc"] = nc
    return nc


def prep_inputs(q_matrix, k_matrix, v_matrix, mask, Wq, bq, Wk, bk, Wv, bv,
                Whq, bhq, Whk, bhk, Whv, bhv, Wo, bo):
    f32 = np.float32
    bf16 = ml_dtypes.bfloat16
    q_matrix = np.asarray(q_matrix, f32)
    k_matrix = np.asarray(k_matrix, f32)
    v_matrix = np.asarray(v_matrix, f32)
    mask = np.asarray(mask)
    sc = f32(1.0 / np.sqrt(HD))

    Wq, Wk, Wv = np.asarray(Wq, f32), np.asarray(Wk, f32), np.asarray(Wv, f32)
    Whq, Whk, Whv = np.asarray(Whq, f32), np.asarray(Whk, f32), np.asarray(Whv, f32)
    bq, bk, bv = np.asarray(bq, f32), np.asarray(bk, f32), np.asarray(bv, f32)
    bhq, bhk, bhv = np.asarray(bhq, f32), np.asarray(bhk, f32), np.asarray(bhv, f32)
    # Fx[h] = Wx @ Whx[h]: one BLAS call via tensordot -> [E(out), H, HD]
    FqH = (np.tensordot(Wq, Whq, axes=([1], [1])) * sc).astype(f32)
    FkH = np.tensordot(Wk, Whk, axes=([1], [1])).astype(f32)
    FvH = np.tensordot(Wv, Whv, axes=([1], [1])).astype(f32)
    bqH = ((np.einsum("e,hed->hd", bq, Whq) + bhq) * sc).astype(f32)
    bkH = (np.einsum("e,hed->hd", bk, Whk) + bhk).astype(f32)
    bvH = (np.einsum("e,hed->hd", bv, Whv) + bhv).astype(f32)
    WoM = np.asarray(Wo, f32)

    onesb = np.ones((1, 128), f32).astype(bf16)
    sel = np.zeros((128, 256), f32)
    for h in range(4):
        sel[32 * h, 64 * h:64 * (h + 1)] = 1.0
    in_maps = []
    for core in range(N_CORES):
        b, g = core // 4, core % 4
        hs = [4 * g + j for j in range(4)]
        Fq_c = np.ascontiguousarray(FqH[:, hs, :].reshape(E, 256)).astype(bf16)
        Fk_c = np.ascontiguousarray(FkH[:, hs, :].reshape(E, 256)).astype(bf16)
        Fv_c = np.zeros((E, 260), f32)
        bfv_c = np.zeros((1, 260), f32)
        for j, h in enumerate(hs):
            Fv_c[:, 65 * j:65 * j + 64] = FvH[:, h, :]
            bfv_c[0, 65 * j:65 * j + 64] = bvH[h]
            bfv_c[0, 65 * j + 64] = 1.0
        bfq_c = np.stack([np.concatenate([bqH[hs[2 * p]], bqH[hs[2 * p + 1]]])
                          for p in range(2)], axis=1)                # [128, 2]
        bfk_c = np.stack([np.concatenate([bkH[hs[2 * p]], bkH[hs[2 * p + 1]]])
                          for p in range(2)], axis=1)
        in_maps.append(dict(
            qT=np.ascontiguousarray(q_matrix[b].T).astype(bf16),
            kT=np.ascontiguousarray(k_matrix[b].T).astype(bf16),
            vT=np.ascontiguousarray(v_matrix[b].T).astype(bf16),
            maskT=np.ascontiguousarray(
                (~mask[b].T).astype(np.float32)).astype(bf16),
            Fq=Fq_c, Fk=Fk_c, Fv=Fv_c.astype(bf16),
            bfq=bfq_c, bfk=bfk_c, bfv=bfv_c.astype(bf16),
            Wo=np.ascontiguousarray(WoM[256 * g:256 * (g + 1), :]).astype(bf16),
            onesb=onesb, sel=sel,
        ))
    return in_maps


def unshard(results, bo):
    bo = np.asarray(bo, np.float32)
    out = np.empty((B, S, E), np.float32)
    for b in range(B):
        acc = results[4 * b]["out_pT"].astype(np.float32).copy()
        for g in range(1, 4):
            acc += results[4 * b + g]["out_pT"]
        out[b] = acc.T + bo
    return out


def kernel(**inputs):
    from concourse.bass_utils import run_bass_kernel_spmd
    nc = build_program()
    in_maps = prep_inputs(**inputs)
    res = run_bass_kernel_spmd(nc, in_maps, list(range(N_CORES)))
    return unshard(res.results, inputs["bo"])
